# revision 22
# baseline (speedup 1.0000x reference)
"""Trainium2 Bass kernel for nn_AttentionBlock (64, 512, 16) / three 8192x8192 Linears.

v4 strategy (8 NeuronCores, single NEFF, one launch):
  fp8(e4m3) weights AND activations for the projection: W' = fp8(W^T*64),
  x' = fp8(x*16), host-packed so every weight-chunk DMA reads 8KB
  contiguous per partition (~line-rate HBM). The PE runs W-stationary
  matmuls (stationary [128,128] chunk, moving x' [128,64]): full-width
  array, ~25MB/core weight traffic (~72us DMA roofline).

  Projections land transposed in PSUM ([cols, batch]). q,k are flipped
  back by PE transposes and shipped through a fp8 AllToAll in [b,(d w)]
  order; v skips the transpose entirely -- its AllToAll payload stays
  [c,b] and part B consumes it that way. A tiny dummy AllToAll up front
  absorbs the ~11us first-collective trigger latency.

  Attention per batch: alphas matmuls write one 4-bank PSUM tile so exp
  is a single N=2048 ScalarE ACTIVATE; softmax-over-q denominators are
  one DVE row-sum + reciprocal. Part B runs the second einsum
  transposed (stationary ea [128,128] chunks, moving rec-scaled v rows
  [128,16]) so the result lands as [w,d] -- sigmoid + residual-add +
  store need no final transpose. Part A for b=0..4 interleaves into the
  Wv chunk loop to fill PE gaps while Wv streams.

  Engine queues are FIFO; emission order is chosen so nothing
  head-of-line-blocks: weight chunks on sync/scalar HWDGE, payloads and
  collective triggers on gpsimd/sync, qkT gathers split gpsimd/scalar
  ahead of the exp stream, vs-scales on gpsimd (DVE drains reduces).
"""

import math

import numpy as np
import ml_dtypes

import concourse.bass as bass
import concourse.bacc as bacc
import concourse.mybir as mybir
import concourse.tile as tile
import concourse.bass_utils as bass_utils

N_CORES = 8
BS, W_DIM, D = 64, 512, 16
K = W_DIM * D            # 8192 contraction dim
CPC = K // N_CORES       # 1024 output cols per core
WPC = W_DIM // N_CORES   # 64 w positions per core
BPC = BS // N_CORES      # 8 batches per core
NKT = K // 128           # 64 k-tiles
CH = 8                   # k-tiles per weight DMA chunk (1MB fp8)
NCH = NKT // CH

SX = 16.0                # host scale on x before fp8
SW = 64.0                # host scale on W before fp8
EPI_SCALE = 1.0 / 128.0  # psum->sbuf dequant; leaves q/k/v scaled by 8
ACT_S = SX * SW * EPI_SCALE                     # = 8
SCALE_EXP = (1.0 / math.sqrt(K)) / (ACT_S * ACT_S)
SIG_SCALE = 1.0 / ACT_S

_CACHE: dict = {}


def _build():
    f32 = mybir.dt.float32
    bf16 = mybir.dt.bfloat16
    f8 = mybir.dt.float8e4
    A2A = "AllToAll"
    BYP = mybir.AluOpType.bypass
    GRP = [list(range(N_CORES))]

    nc = bacc.Bacc("TRN2", target_bir_lowering=False, debug=False,
                   num_devices=N_CORES)

    xq_d = nc.dram_tensor("xq", [128, NKT * BS], f8, kind="ExternalInput")
    w_d = [nc.dram_tensor(n, [NCH * 128, CH * CPC], f8, kind="ExternalInput")
           for n in ("wq", "wk", "wv")]
    b_d = [nc.dram_tensor(n, [1, CPC], bf16, kind="ExternalInput")
           for n in ("bq", "bk", "bv")]
    id_d = nc.dram_tensor("ident", [128, 128], bf16, kind="ExternalInput")
    xtp_d = nc.dram_tensor("xtp", [BPC, W_DIM, D], f32, kind="ExternalInput")
    out_d = nc.dram_tensor("out", [BPC, W_DIM, D], f32, kind="ExternalOutput")

    with tile.TileContext(nc) as tc:
        with (
            tc.tile_pool(name="constp", bufs=1) as constp,
            tc.tile_pool(name="sbp", bufs=1) as sbp,
            tc.tile_pool(name="dramp", bufs=1, space="DRAM") as dramp,
            tc.tile_pool(name="wp", bufs=6) as wp,
            tc.tile_pool(name="epi", bufs=2) as epi,
            tc.tile_pool(name="attp", bufs=3) as attp,
            tc.tile_pool(name="keepp", bufs=1) as keepp,
            tc.tile_pool(name="accp", bufs=1, space="PSUM") as accp,
            tc.tile_pool(name="tps", bufs=1, space="PSUM") as tps,
            tc.tile_pool(name="attps", bufs=2, space="PSUM") as attps,
            tc.tile_pool(name="rtps", bufs=2, space="PSUM") as rtps,
        ):
            # ---- constants ----
            xq_sb = constp.tile([128, NKT, BS], f8)
            nc.sync.dma_start(
                xq_sb[:], xq_d[:, :].rearrange("p (kt b) -> p kt b", kt=NKT))
            ident = constp.tile([128, 128], bf16)
            nc.scalar.dma_start(ident[:], id_d[:, :])
            ones = constp.tile([1, BS], bf16)
            nc.gpsimd.memset(ones[:], 1.0)
            b_sb = []
            for t in range(3):
                bt = constp.tile([1, CPC], bf16, name=f"bias{t}")
                nc.scalar.dma_start(bt[:], b_d[t][:, :])
                b_sb.append(bt)

            # ---- persistent activations ----
            qk_sb = sbp.tile([BS, 2, D, WPC], f8, name="qk_sb")
            a2aq_in = dramp.tile([N_CORES, BPC, CPC], f8,
                                 tag="a2aq_in", name="a2aq_in")
            a2aq_out = dramp.tile([N_CORES, BPC, CPC], f8,
                                  tag="a2aq_out", name="a2aq_out")
            a2ak_in = dramp.tile([N_CORES, BPC, CPC], f8,
                                 tag="a2ak_in", name="a2ak_in")
            a2ak_out = dramp.tile([N_CORES, BPC, CPC], f8,
                                  tag="a2ak_out", name="a2ak_out")
            a2a1_io = [(a2aq_in, a2aq_out), (a2ak_in, a2ak_out)]
            a2a2_in = dramp.tile([N_CORES, CPC, BPC], f8,
                                 tag="a2a2_in", name="a2a2_in")
            a2a2_out = dramp.tile([N_CORES, CPC, BPC], f8,
                                  tag="a2a2_out", name="a2a2_out")

            def mm_chunk(acc, m, wt):
                for j in range(CH):
                    kt = m * CH + j
                    for cc in range(8):
                        nc.tensor.matmul(
                            acc[:, cc, :],
                            wt[:, j, cc * 128:(cc + 1) * 128],
                            xq_sb[:, kt, :],
                            start=(kt == 0), stop=False)

            def bias_mms(t, acc):
                for cc in range(8):
                    nc.tensor.matmul(
                        acc[:, cc, :],
                        b_sb[t][:, cc * 128:(cc + 1) * 128],
                        ones[:], start=False, stop=True)

            def epilogue_qk(t, acc):
                # psum [c,b] -> (dequant, bf16) -> PE transpose -> [b,c]
                sb = epi.tile([128, 8, BS], bf16, tag="episb", name=f"esb{t}")
                nc.vector.tensor_scalar_mul(sb[:], acc[:], EPI_SCALE)
                ps = tps.tile([BS, CPC], bf16, tag="tpsum", name=f"tps{t}")
                for cc in range(8):
                    nc.tensor.transpose(
                        ps[:, cc * 128:(cc + 1) * 128], sb[:, cc, :],
                        ident[:])
                # c index is (w d); store [b, d, w] for the a2a payload
                nc.vector.tensor_copy(
                    qk_sb[:, t, :, :],
                    ps[:, :].rearrange("b (w d) -> b d w", d=D))

            # ================ phase 1: q then k ================
            for t in range(2):
                acc = accp.tile([128, 8, BS], f32, tag="acc", name=f"acc{t}")
                rings = [nc.scalar, nc.sync] if t == 0 else [nc.sync, nc.scalar]
                for m in range(NCH):
                    wt = wp.tile([128, CH, CPC], f8, tag="w", name=f"wt{t}")
                    rings[m % 2].dma_start(
                        wt[:].rearrange("p j c -> p (j c)"),
                        w_d[t][:, :].rearrange(
                            "(m p) jc -> m p jc", p=128)[m])
                    mm_chunk(acc, m, wt)
                bias_mms(t, acc)
                epilogue_qk(t, acc)
                tin, tout = a2a1_io[t]
                nc.gpsimd.dma_start(
                    tin[:, :, :].rearrange("j b c -> (j b) c"),
                    qk_sb[:, t, :, :].rearrange("b d w -> b (d w)"))
                nc.gpsimd.collective_compute(
                    A2A, BYP, replica_groups=GRP,
                    ins=[tin.opt()], outs=[tout.opt()])

            # ================ part A (per batch) ================
            ea_tiles = {}
            rec_tiles = {}

            def emit_part_a(b):
                # kw-halves through double-buffered 2-bank psum tiles so
                # the exp stream never serializes behind its own matmuls
                qkT = qkT_tiles[b]
                ea = keepp.tile([128, 4, 512], bf16, tag=f"ea{b}",
                                name=f"ea{b}")
                den = keepp.tile([128, 4], f32, tag=f"den{b}", name=f"den{b}")
                for h2 in range(2):
                    aT = attps.tile([128, 2, 512], f32, tag="aT", name="aT")
                    for k2 in range(2):
                        kw = 2 * h2 + k2
                        nc.tensor.matmul(
                            aT[:, k2, :], qkT[:, 1, 2 * kw:2 * kw + 2, :],
                            qkT[:, 0, :, :], start=True, stop=True)
                    nc.scalar.activation(
                        ea[:, 2 * h2:2 * h2 + 2, :], aT[:],
                        mybir.ActivationFunctionType.Exp, scale=SCALE_EXP)
                    nc.vector.tensor_reduce(
                        den[:, 2 * h2:2 * h2 + 2], ea[:, 2 * h2:2 * h2 + 2, :],
                        axis=mybir.AxisListType.X, op=mybir.AluOpType.add)
                rec = keepp.tile([128, 4], f32, tag=f"rec{b}", name=f"rec{b}")
                nc.vector.reciprocal(rec[:], den[:])
                ea_tiles[b] = ea
                rec_tiles[b] = rec

            # ================ phase 2: v (sync ring only); part A for
            # b=0..4 interleaved into the chunk stream ================
            accv = accp.tile([128, 8, BS], f32, tag="acc", name="accv")
            for m in range(NCH):
                wt = wp.tile([128, CH, CPC], f8, tag="w", name="wtv")
                nc.sync.dma_start(
                    wt[:].rearrange("p j c -> p (j c)"),
                    w_d[2][:, :].rearrange(
                        "(m p) jc -> m p jc", p=128)[m])
                mm_chunk(accv, m, wt)
            bias_mms(2, accv)
            # v epilogue: dequant straight to fp8 in [c -> (j,cc), b']
            # payload order (ScalarE: DVE is mid-reduce, gpsimd can't
            # read PSUM); no transpose needed for v
            vT = epi.tile([128, 8, 8, 8], f8, tag="vT", name="vT")
            nc.scalar.activation(
                vT[:, :, :, :].rearrange("p j cc b -> p cc j b"),
                accv[:, :, :].rearrange("p cc (j b) -> p cc j b", j=8),
                mybir.ActivationFunctionType.Copy, scale=EPI_SCALE)
            # payload: one DMA [j, (cc p), b'] <- vT[p, j, cc, b']
            nc.sync.dma_start(
                a2a2_in[:, :, :].rearrange("j (cc p) b -> p j cc b", p=128),
                vT[:, :, :, :])
            nc.gpsimd.collective_compute(
                A2A, BYP, replica_groups=GRP,
                ins=[a2a2_in.opt()], outs=[a2a2_out.opt()])

            # hoisted qkT gathers for every batch, split across the
            # gpsimd + scalar rings (all ready the moment A2A#1 lands,
            # ahead of the exp stream in scalar's FIFO)
            qkT_tiles = []
            for b in range(BPC):
                qkT = keepp.tile([D, 2, N_CORES, WPC], f8, name=f"qkT{b}")
                for t2 in range(2):
                    eng = nc.gpsimd if (2 * b + t2) % 2 == 0 else nc.scalar
                    eng.dma_start(
                        qkT[:, t2, :, :],
                        a2a1_io[t2][1][:, b, :].rearrange(
                            "i (d w) -> d i w", d=D))
                qkT_tiles.append(qkT)

            for b in range(BPC):
                emit_part_a(b)

            # ================ part B ================
            # v rows [h*64+w, b, kw, d] from the [i, c, b'] a2a layout;
            # 8 gathers split across both HWDGE rings
            vt_all = keepp.tile([128, 4, D, BPC], f8, name="vt_all")
            for kw in range(4):
                for h in range(2):
                    eng = nc.sync if h == 0 else nc.scalar
                    eng.dma_start(
                        vt_all[64 * h:64 * h + 64, kw, :, :],
                        a2a2_out[2 * kw + h].rearrange(
                            "(w d) b -> w d b", d=D))
            # residual input, already [w, d] per batch
            xb_tiles = []
            for b in range(BPC):
                xb = keepp.tile([128, 4, D], f32, name=f"xb{b}")
                nc.gpsimd.dma_start(
                    xb[:], xtp_d[b].rearrange("(qc p) d -> p qc d", p=128))
                xb_tiles.append(xb)
            for b in range(BPC):
                vs = attp.tile([128, 4, D], bf16, tag="vs", name="vs")
                for kw in range(4):
                    nc.vector.tensor_scalar_mul(
                        vs[:, kw, :], vt_all[:, kw, :, b],
                        rec_tiles[b][:, kw:kw + 1])
                # transposed second einsum: out[q, d] so no final
                # transpose; stationary ea chunks stream only 16 cols
                rtt = rtps.tile([128, 4, D], f32, tag="rtt", name="rtt")
                for qc in range(4):
                    for kw in range(4):
                        nc.tensor.matmul(
                            rtt[:, qc, :],
                            ea_tiles[b][:, kw, 128 * qc:128 * (qc + 1)],
                            vs[:, kw, :],
                            start=(kw == 0), stop=(kw == 3))
                sg = attp.tile([128, 4, D], f32, tag="sg", name="sg")
                nc.scalar.activation(
                    sg[:], rtt[:], mybir.ActivationFunctionType.Sigmoid,
                    scale=SIG_SCALE)
                oo = attp.tile([128, 4, D], f32, tag="oo", name="oo")
                nc.vector.tensor_add(oo[:], sg[:], xb_tiles[b][:])
                nc.sync.dma_start(
                    out_d[b].rearrange("(qc p) d -> p qc d", p=128), oo[:])

    nc.compile()
    return nc


def _prep_in_maps(x_in, Wq, bq, Wk, bk, Wv, bv, use_bf16=None):
    f8 = ml_dtypes.float8_e4m3
    bf = ml_dtypes.bfloat16

    x_flat = np.asarray(x_in, np.float32).reshape(BS, K)
    # swizzled x^T: [128 p, kt, b], scaled and quantized to fp8
    xq = np.ascontiguousarray(
        x_flat.T.reshape(NKT, 128, BS).transpose(1, 0, 2)
    ).reshape(128, NKT * BS)
    xq = np.clip(xq * SX, -240, 240).astype(f8)

    def pack_w(WT_core):
        # [K, CPC] -> [m, p, j, c] so each chunk is 8KB/partition contiguous
        q = np.clip(WT_core * SW, -240, 240).astype(f8)
        return np.ascontiguousarray(
            q.reshape(NCH, CH, 128, CPC).transpose(0, 2, 1, 3)
        ).reshape(NCH * 128, CH * CPC)

    ws = [np.ascontiguousarray(np.asarray(W, np.float32).T)
          for W in (Wq, Wk, Wv)]
    bs = [np.asarray(b, np.float32).reshape(1, K) * (SX * SW)
          for b in (bq, bk, bv)]
    x3 = np.asarray(x_in, np.float32)                       # (BS, W, D)
    ident = np.eye(128, dtype=np.float32).astype(bf)

    in_maps = []
    for c in range(N_CORES):
        cs = slice(CPC * c, CPC * (c + 1))
        m = {
            "xq": xq,
            "wq": pack_w(ws[0][:, cs]),
            "wk": pack_w(ws[1][:, cs]),
            "wv": pack_w(ws[2][:, cs]),
            "bq": bs[0][:, cs].astype(bf),
            "bk": bs[1][:, cs].astype(bf),
            "bv": bs[2][:, cs].astype(bf),
            "ident": ident,
            "xtp": np.ascontiguousarray(x3[BPC * c:BPC * (c + 1)]),
        }
        in_maps.append(m)
    return in_maps


def _assemble(results):
    out = np.empty((BS, W_DIM, D), np.float32)
    for c in range(N_CORES):
        out[BPC * c:BPC * (c + 1)] = results[c]["out"]       # (BPC, W, D)
    return out


USE_BF16 = True  # kept for timing.py compat; unused


def get_nc(use_bf16=None):
    if "nc" not in _CACHE:
        _CACHE["nc"] = _build()
    return _CACHE["nc"]


def kernel(x_in, Wq, bq, Wk, bk, Wv, bv):
    nc = get_nc()
    in_maps = _prep_in_maps(x_in, Wq, bq, Wk, bk, Wv, bv)
    res = bass_utils.run_bass_kernel_spmd(
        nc, in_maps, core_ids=list(range(N_CORES)))
    return _assemble(res.results)
